# revision 3
# baseline (speedup 1.0000x reference)
"""AmpNorm Trainium2 kernel: FFT2 amplitude normalization via DFT matmuls.

Distributed data-parallel over batch across 8 NeuronCores; per-device
amplitude sums are all-reduced so the EMA/running-amplitude update is
replicated. Self-contained: hardcodes shapes [32,3,384,384] / [3,384,384].
"""

import sys

if "/opt/trn_rl_repo" not in sys.path:
    sys.path.insert(0, "/opt/trn_rl_repo")

import numpy as np

import concourse.bass as bass
import concourse.mybir as mybir
from concourse import bacc
from concourse.tile import TileContext
from concourse.bass_utils import run_bass_kernel_spmd

N_CORES = 8
B_FULL = 32
CH = 3
N = 384
NT = N // 128          # 3 partition tiles per 384 axis
KH = N // 2 + 1        # 193 Hermitian half-plane columns
KP = 256               # stage-2 free-dim padding (fp32r needs N>=256 for full rate)
KT1 = 128              # stage-3/4 k2 tile sizes
KT2 = KH - 128         # 65
B_LOC = B_FULL // N_CORES  # 4 samples per core
MOMENTUM = 0.1

F32 = mybir.dt.float32
F32R = mybir.dt.float32r

_COMPILED = {}


def _dft_constants():
    j = np.arange(N)
    ang = 2.0 * np.pi * np.outer(j, j) / N
    Cm = np.cos(ang).astype(np.float32)
    Sm = np.sin(ang).astype(np.float32)
    Snm = (-Sm).astype(np.float32)
    w = np.full(KH, 2.0, dtype=np.float32)
    w[0] = 1.0
    w[KH - 1] = 1.0
    C4 = np.zeros((2 * 128, N), dtype=np.float32)
    Sn4 = np.zeros((2 * 128, N), dtype=np.float32)
    C4[:KH] = Cm[:KH, :] * (w[:, None] / (N * N))
    Sn4[:KH] = Snm[:KH, :] * (w[:, None] / (N * N))
    return Cm, Sm, Snm, C4, Sn4


def _build(b_coef: float):
    """Build + compile the SPMD graph. b_coef scales the all-reduced amp sum
    (b/32 for the EMA blend); the additive bias tensor is a runtime input."""
    nc = bacc.Bacc("TRN2", target_bir_lowering=False, debug=False,
                   num_devices=N_CORES)

    x_ext = nc.dram_tensor("x", [B_LOC, CH, N, N], F32R, kind="ExternalInput")
    cm_ext = nc.dram_tensor("cm", [N, N], F32R, kind="ExternalInput")
    sm_ext = nc.dram_tensor("sm", [N, N], F32R, kind="ExternalInput")
    snm_ext = nc.dram_tensor("snm", [N, N], F32R, kind="ExternalInput")
    c4_ext = nc.dram_tensor("c4", [256, N], F32R, kind="ExternalInput")
    sn4_ext = nc.dram_tensor("sn4", [256, N], F32R, kind="ExternalInput")
    bias_ext = nc.dram_tensor("bias", [CH, NT, 128, KH], F32, kind="ExternalInput")
    out_ext = nc.dram_tensor("out", [B_LOC, CH, N, N], F32, kind="ExternalOutput")

    ar_in = nc.dram_tensor("ar_in", [CH, NT, 128, KH], F32)
    ar_out = nc.dram_tensor("ar_out", [CH, NT, 128, KH], F32, addr_space="Shared")

    AluOp = mybir.AluOpType
    Act = mybir.ActivationFunctionType

    with TileContext(nc) as tc:
        with tc.tile_pool(name="sb", bufs=1) as sb, \
             tc.tile_pool(name="ps", bufs=2, space="PSUM") as ps:

            # --- constants ---
            c_t = sb.tile([128, NT, N], F32R, tag="c_t")
            s_t = sb.tile([128, NT, N], F32R, tag="s_t")
            sn_t = sb.tile([128, NT, N], F32R, tag="sn_t")
            c4_t = sb.tile([128, 2, N], F32R, tag="c4_t")
            sn4_t = sb.tile([128, 2, N], F32R, tag="sn4_t")
            nc.sync.dma_start(out=c_t[:], in_=cm_ext.ap().rearrange("(t p) w -> p t w", p=128))
            nc.sync.dma_start(out=s_t[:], in_=sm_ext.ap().rearrange("(t p) w -> p t w", p=128))
            nc.sync.dma_start(out=sn_t[:], in_=snm_ext.ap().rearrange("(t p) w -> p t w", p=128))
            nc.sync.dma_start(out=c4_t[:], in_=c4_ext.ap().rearrange("(t p) w -> p t w", p=128))
            nc.sync.dma_start(out=sn4_t[:], in_=sn4_ext.ap().rearrange("(t p) w -> p t w", p=128))

            bias_sb = []
            as_sb = []
            for c in range(CH):
                bt = sb.tile([128, NT, KH], F32, tag=f"bias{c}", name=f"bias_sb{c}")
                nc.sync.dma_start(out=bt[:], in_=bias_ext[c].rearrange("t p k -> p t k"))
                bias_sb.append(bt)
                at = sb.tile([128, NT, KH], F32, tag=f"as{c}", name=f"as_sb{c}")
                as_sb.append(at)

            # persistent per-image unit-spectrum storage
            e_r = {}
            e_i = {}
            for c in range(CH):
                for s in range(B_LOC):
                    img = c * B_LOC + s
                    e_r[img] = sb.tile([128, NT, KH], F32, tag=f"er{img}", name=f"er{img}")
                    e_i[img] = sb.tile([128, NT, KH], F32, tag=f"ei{img}", name=f"ei{img}")

            def emit_group(psum_ap, mms):
                """Emit an accumulation group: list of (lhsT, rhs)."""
                last = len(mms) - 1
                for i, (lh, rh) in enumerate(mms):
                    nc.tensor.matmul(psum_ap, lh, rh, start=(i == 0), stop=(i == last))

            # ---------------- phase 1: forward FFT + amp/E, per channel ----
            for c in range(CH):
                for s in range(B_LOC):
                    img = c * B_LOC + s
                    x_sb = sb.tile([128, NT, N], F32R, tag="x_sb", bufs=3)
                    nc.sync.dma_start(
                        out=x_sb[:],
                        in_=x_ext[s, c].rearrange("(t p) w -> p t w", p=128))

                    b_r = sb.tile([128, NT, N], F32R, tag="b_r", bufs=2)
                    b_i = sb.tile([128, NT, N], F32R, tag="b_i", bufs=2)
                    for m in range(NT):
                        bps_r = ps.tile([128, N], F32, tag="psA", name="bps_r")
                        bps_i = ps.tile([128, N], F32, tag="psB", name="bps_i")
                        ms = slice(m * 128, (m + 1) * 128)
                        emit_group(bps_r[:, :],
                                   [(x_sb[:, k, ms], c_t[:, k, :]) for k in range(NT)])
                        emit_group(bps_i[:, :],
                                   [(x_sb[:, k, ms], sn_t[:, k, :]) for k in range(NT)])
                        nc.vector.tensor_copy(b_r[:, m, :], bps_r[:, :])
                        nc.vector.tensor_copy(b_i[:, m, :], bps_i[:, :])

                    for m2 in range(NT):
                        xr_ps = ps.tile([128, KP], F32, tag="psC", name="xr_ps")
                        xi_ps = ps.tile([128, KP], F32, tag="psD", name="xi_ps")
                        ms = slice(m2 * 128, (m2 + 1) * 128)
                        emit_group(
                            xr_ps[:, :],
                            [(b_r[:, k, ms], c_t[:, k, 0:KP]) for k in range(NT)]
                            + [(b_i[:, k, ms], s_t[:, k, 0:KP]) for k in range(NT)])
                        emit_group(
                            xi_ps[:, :],
                            [(b_i[:, k, ms], c_t[:, k, 0:KP]) for k in range(NT)]
                            + [(b_r[:, k, ms], sn_t[:, k, 0:KP]) for k in range(NT)])

                        t1 = sb.tile([128, KH], F32, tag="t1", bufs=2)
                        t2 = sb.tile([128, KH], F32, tag="t2", bufs=2)
                        nc.scalar.activation(t1[:, :], xr_ps[:, 0:KH], Act.Square)
                        nc.scalar.activation(t2[:, :], xi_ps[:, 0:KH], Act.Square)
                        nc.vector.tensor_add(t1[:, :], t1[:, :], t2[:, :])
                        if s == 0:
                            amp_ap = as_sb[c][:, m2, :]
                        else:
                            amp_t = sb.tile([128, KH], F32, tag="amp_t", bufs=2)
                            amp_ap = amp_t[:, :]
                        nc.scalar.activation(amp_ap, t1[:, :], Act.Sqrt)
                        rec = sb.tile([128, KH], F32, tag="rec", bufs=2)
                        nc.vector.reciprocal(rec[:, :], amp_ap)
                        if s != 0:
                            nc.vector.tensor_add(as_sb[c][:, m2, :],
                                                 as_sb[c][:, m2, :], amp_ap)
                        nc.vector.tensor_mul(e_r[img][:, m2, :], xr_ps[:, 0:KH], rec[:, :])
                        nc.vector.tensor_mul(e_i[img][:, m2, :], xi_ps[:, 0:KH], rec[:, :])

                # channel done -> all-reduce this channel's amp sum
                nc.sync.dma_start(out=ar_in[c].rearrange("t p k -> p t k"),
                                  in_=as_sb[c][:])
                nc.gpsimd.collective_compute(
                    "AllReduce",
                    AluOp.add,
                    replica_groups=[list(range(N_CORES))],
                    ins=[ar_in[c].opt()],
                    outs=[ar_out[c].opt()],
                )

            # ---------------- phase 2+3: scale + inverse FFT ---------------
            for c in range(CH):
                # M = b_coef * global_amp_sum + bias   (in place in as_sb[c])
                nc.sync.dma_start(out=as_sb[c][:],
                                  in_=ar_out[c].rearrange("t p k -> p t k"))
                nc.vector.scalar_tensor_tensor(
                    out=as_sb[c][:], in0=as_sb[c][:], scalar=float(b_coef),
                    in1=bias_sb[c][:], op0=AluOp.mult, op1=AluOp.add)

                for s in range(B_LOC):
                    img = c * B_LOC + s
                    yr = sb.tile([128, NT, KH], F32R, tag="yr", bufs=2)
                    yi = sb.tile([128, NT, KH], F32R, tag="yi", bufs=2)
                    nc.vector.tensor_mul(yr[:], e_r[img][:], as_sb[c][:])
                    nc.vector.tensor_mul(yi[:], e_i[img][:], as_sb[c][:])

                    p_r0 = sb.tile([128, N], F32R, tag="p_r0", bufs=2)
                    p_r1 = sb.tile([128, N], F32R, tag="p_r1", bufs=2)
                    p_i0 = sb.tile([128, N], F32R, tag="p_i0", bufs=2)
                    p_i1 = sb.tile([128, N], F32R, tag="p_i1", bufs=2)
                    for m3 in range(2):
                        mm = 128 if m3 == 0 else KT2
                        k2s = slice(m3 * 128, m3 * 128 + mm)
                        pr_ps = ps.tile([128, N], F32, tag="psA", name="pr_ps")
                        pi_ps = ps.tile([128, N], F32, tag="psB", name="pi_ps")
                        emit_group(
                            pr_ps[0:mm, :],
                            [(yr[:, k, k2s], c_t[:, k, :]) for k in range(NT)]
                            + [(yi[:, k, k2s], sn_t[:, k, :]) for k in range(NT)])
                        emit_group(
                            pi_ps[0:mm, :],
                            [(yi[:, k, k2s], c_t[:, k, :]) for k in range(NT)]
                            + [(yr[:, k, k2s], s_t[:, k, :]) for k in range(NT)])
                        pr_sb = p_r0 if m3 == 0 else p_r1
                        pi_sb = p_i0 if m3 == 0 else p_i1
                        nc.vector.tensor_copy(pr_sb[0:mm, :], pr_ps[0:mm, :])
                        nc.vector.tensor_copy(pi_sb[0:mm, :], pi_ps[0:mm, :])

                    for mh in range(NT):
                        hs = slice(mh * 128, (mh + 1) * 128)
                        o_ps = ps.tile([128, N], F32, tag="psC", name="o_ps")
                        emit_group(
                            o_ps[:, :],
                            [(p_r0[:, hs], c4_t[:, 0, :]),
                             (p_r1[0:KT2, hs], c4_t[0:KT2, 1, :]),
                             (p_i0[:, hs], sn4_t[:, 0, :]),
                             (p_i1[0:KT2, hs], sn4_t[0:KT2, 1, :])])
                        out_sb = sb.tile([128, N], F32, tag="out_sb", bufs=3)
                        nc.scalar.copy(out_sb[:, :], o_ps[:, :])
                        nc.sync.dma_start(out=out_ext[s, c, hs, :], in_=out_sb[:, :])

    nc.compile()
    return nc


def _get_nc(b_coef: float):
    key = round(float(b_coef), 9)
    if key not in _COMPILED:
        _COMPILED[key] = _build(b_coef)
    return _COMPILED[key]


def kernel(x: np.ndarray, running_amp: np.ndarray) -> np.ndarray:
    x = np.ascontiguousarray(x, dtype=np.float32)
    running_amp = np.asarray(running_amp, dtype=np.float32)

    if float(np.sum(running_amp)) == 0.0:
        a_coef, b_coef = 0.0, 1.0
    else:
        a_coef, b_coef = 1.0 - MOMENTUM, MOMENTUM

    Cm, Sm, Snm, C4, Sn4 = _dft_constants()
    bias = (a_coef * running_amp[:, :, 0:KH]).reshape(CH, NT, 128, KH)
    bias = np.ascontiguousarray(bias, dtype=np.float32)

    nc = _get_nc(b_coef / B_FULL)

    in_maps = []
    for i in range(N_CORES):
        in_maps.append({
            "x": x[i * B_LOC:(i + 1) * B_LOC],
            "cm": Cm, "sm": Sm, "snm": Snm, "c4": C4, "sn4": Sn4,
            "bias": bias,
        })
    res = run_bass_kernel_spmd(nc, in_maps, list(range(N_CORES)))
    out = np.concatenate([res.results[i]["out"] for i in range(N_CORES)], axis=0)
    return out.astype(np.float32)


# revision 8
# speedup vs baseline: 1.1294x; 1.1294x over previous
"""AmpNorm Trainium2 kernel: FFT2 amplitude normalization via DFT matmuls.

Distributed data-parallel over batch across 8 NeuronCores; per-device
amplitude sums are all-reduced so the EMA/running-amplitude update is
replicated. Self-contained: hardcodes shapes [32,3,384,384] / [3,384,384].
"""

import sys

if "/opt/trn_rl_repo" not in sys.path:
    sys.path.insert(0, "/opt/trn_rl_repo")

import numpy as np

import concourse.bass as bass
import concourse.mybir as mybir
from concourse import bacc
from concourse.tile import TileContext
from concourse.bass_utils import run_bass_kernel_spmd

N_CORES = 8
B_FULL = 32
CH = 3
N = 384
NT = N // 128          # 3 partition tiles per 384 axis
KH = N // 2 + 1        # 193 Hermitian half-plane columns
KP = 256               # stage-2 free-dim padding (fp32r needs N>=256 for full rate)
KT1 = 128              # stage-3/4 k2 tile sizes
KT2 = KH - 128         # 65
B_LOC = B_FULL // N_CORES  # 4 samples per core
MOMENTUM = 0.1

F32 = mybir.dt.float32
F32R = mybir.dt.float32r

_COMPILED = {}


def _dft_constants():
    j = np.arange(N)
    ang = 2.0 * np.pi * np.outer(j, j) / N
    Cm = np.cos(ang).astype(np.float32)
    Sm = np.sin(ang).astype(np.float32)
    Snm = (-Sm).astype(np.float32)
    w = np.full(KH, 2.0, dtype=np.float32)
    w[0] = 1.0
    w[KH - 1] = 1.0
    C4 = np.zeros((2 * 128, N), dtype=np.float32)
    Sn4 = np.zeros((2 * 128, N), dtype=np.float32)
    C4[:KH] = Cm[:KH, :] * (w[:, None] / (N * N))
    Sn4[:KH] = Snm[:KH, :] * (w[:, None] / (N * N))
    return Cm, Sm, Snm, C4, Sn4


def _build(b_coef: float):
    """Build + compile the SPMD graph. b_coef scales the all-reduced amp sum
    (b/32 for the EMA blend); the additive bias tensor is a runtime input."""
    nc = bacc.Bacc("TRN2", target_bir_lowering=False, debug=False,
                   num_devices=N_CORES)

    x_ext = nc.dram_tensor("x", [B_LOC, CH, N, N], F32R, kind="ExternalInput")
    cm_ext = nc.dram_tensor("cm", [N, N], F32R, kind="ExternalInput")
    sm_ext = nc.dram_tensor("sm", [N, N], F32R, kind="ExternalInput")
    snm_ext = nc.dram_tensor("snm", [N, N], F32R, kind="ExternalInput")
    c4_ext = nc.dram_tensor("c4", [256, N], F32R, kind="ExternalInput")
    sn4_ext = nc.dram_tensor("sn4", [256, N], F32R, kind="ExternalInput")
    bias_ext = nc.dram_tensor("bias", [CH, NT, 128, KH], F32, kind="ExternalInput")
    out_ext = nc.dram_tensor("out", [B_LOC, CH, N, N], F32, kind="ExternalOutput")

    ar_in = nc.dram_tensor("ar_in", [CH, NT, 128, KH], F32)
    ar_out = nc.dram_tensor("ar_out", [CH, NT, 128, KH], F32, addr_space="Shared")

    AluOp = mybir.AluOpType
    Act = mybir.ActivationFunctionType

    with TileContext(nc) as tc:
        with tc.tile_pool(name="sb", bufs=1) as sb, \
             tc.tile_pool(name="ps", bufs=2, space="PSUM") as ps:

            # --- constants (c/sn first: they gate the first stage-1 MMs;
            # c4/sn4/bias are deferred until after phase 1's emission) ---
            c_t = sb.tile([128, NT, N], F32R, tag="c_t")
            s_t = sb.tile([128, NT, N], F32R, tag="s_t")
            sn_t = sb.tile([128, NT, N], F32R, tag="sn_t")
            c4_t = sb.tile([128, 2, N], F32R, tag="c4_t")
            sn4_t = sb.tile([128, 2, N], F32R, tag="sn4_t")
            nc.sync.dma_start(out=c_t[:], in_=cm_ext.ap().rearrange("(t p) w -> p t w", p=128))
            nc.sync.dma_start(out=sn_t[:], in_=snm_ext.ap().rearrange("(t p) w -> p t w", p=128))
            nc.sync.dma_start(out=s_t[:], in_=sm_ext.ap().rearrange("(t p) w -> p t w", p=128))

            bias_sb = []
            as_sb = []
            for c in range(CH):
                bt = sb.tile([128, NT, KH], F32, tag=f"bias{c}", name=f"bias_sb{c}")
                bias_sb.append(bt)
                at = sb.tile([128, NT, KH], F32, tag=f"as{c}", name=f"as_sb{c}")
                as_sb.append(at)

            # persistent per-image unit-spectrum storage
            e_r = {}
            e_i = {}
            for c in range(CH):
                for s in range(B_LOC):
                    img = c * B_LOC + s
                    e_r[img] = sb.tile([128, NT, KH], F32, tag=f"er{img}", name=f"er{img}")
                    e_i[img] = sb.tile([128, NT, KH], F32, tag=f"ei{img}", name=f"ei{img}")

            def emit_group(psum_ap, mms):
                """Emit an accumulation group: list of (lhsT, rhs)."""
                last = len(mms) - 1
                for i, (lh, rh) in enumerate(mms):
                    nc.tensor.matmul(psum_ap, lh, rh, start=(i == 0), stop=(i == last))

            # ---------------- phase 1: forward FFT + amp/E, per channel ----
            for c in range(CH):
                for s in range(B_LOC):
                    img = c * B_LOC + s
                    x_sb = sb.tile([128, NT, N], F32R, tag="x_sb", bufs=3)
                    nc.sync.dma_start(
                        out=x_sb[:],
                        in_=x_ext[s, c].rearrange("(t p) w -> p t w", p=128))

                    b_r = sb.tile([128, NT, N], F32R, tag="b_r", bufs=2)
                    b_i = sb.tile([128, NT, N], F32R, tag="b_i", bufs=2)
                    for m in range(NT):
                        bps_r = ps.tile([128, N], F32, tag="psA", name="bps_r")
                        bps_i = ps.tile([128, N], F32, tag="psB", name="bps_i")
                        ms = slice(m * 128, (m + 1) * 128)
                        emit_group(bps_r[:, :],
                                   [(x_sb[:, k, ms], c_t[:, k, :]) for k in range(NT)])
                        emit_group(bps_i[:, :],
                                   [(x_sb[:, k, ms], sn_t[:, k, :]) for k in range(NT)])
                        nc.vector.tensor_copy(b_r[:, m, :], bps_r[:, :])
                        nc.scalar.copy(b_i[:, m, :], bps_i[:, :])

                    for m2 in range(NT):
                        xr_ps = ps.tile([128, KP], F32, tag="psC", name="xr_ps")
                        xi_ps = ps.tile([128, KP], F32, tag="psD", name="xi_ps")
                        ms = slice(m2 * 128, (m2 + 1) * 128)
                        emit_group(
                            xr_ps[:, :],
                            [(b_r[:, k, ms], c_t[:, k, 0:KP]) for k in range(NT)]
                            + [(b_i[:, k, ms], s_t[:, k, 0:KP]) for k in range(NT)])
                        emit_group(
                            xi_ps[:, :],
                            [(b_i[:, k, ms], c_t[:, k, 0:KP]) for k in range(NT)]
                            + [(b_r[:, k, ms], sn_t[:, k, 0:KP]) for k in range(NT)])

                        t1 = sb.tile([128, KH], F32, tag="t1", bufs=2)
                        t2 = sb.tile([128, KH], F32, tag="t2", bufs=2)
                        nc.scalar.activation(t1[:, :], xr_ps[:, 0:KH], Act.Square)
                        nc.scalar.activation(t2[:, :], xi_ps[:, 0:KH], Act.Square)
                        nc.vector.tensor_add(t1[:, :], t1[:, :], t2[:, :])
                        if s == 0:
                            amp_ap = as_sb[c][:, m2, :]
                        else:
                            amp_t = sb.tile([128, KH], F32, tag="amp_t", bufs=2)
                            amp_ap = amp_t[:, :]
                        nc.scalar.activation(amp_ap, t1[:, :], Act.Sqrt)
                        rec = sb.tile([128, KH], F32, tag="rec", bufs=2)
                        nc.vector.reciprocal_approx_fast(rec[:, :], amp_ap)
                        if s != 0:
                            nc.vector.tensor_add(as_sb[c][:, m2, :],
                                                 as_sb[c][:, m2, :], amp_ap)
                        nc.vector.tensor_mul(e_r[img][:, m2, :], xr_ps[:, 0:KH], rec[:, :])
                        nc.vector.tensor_mul(e_i[img][:, m2, :], xi_ps[:, 0:KH], rec[:, :])

                # channel done -> all-reduce this channel's amp sum
                # (AR bounce traffic rides the gpsimd DMA queue so it is not
                # stuck behind x/out transfers on the sync queue)
                nc.gpsimd.dma_start(out=ar_in[c].rearrange("t p k -> p t k"),
                                    in_=as_sb[c][:])
                nc.gpsimd.collective_compute(
                    "AllReduce",
                    AluOp.add,
                    replica_groups=[list(range(N_CORES))],
                    ins=[ar_in[c].opt()],
                    outs=[ar_out[c].opt()],
                )
                nc.gpsimd.dma_start(out=as_sb[c][:],
                                    in_=ar_out[c].rearrange("t p k -> p t k"))

            # deferred constants for the inverse stages
            nc.sync.dma_start(out=c4_t[:], in_=c4_ext.ap().rearrange("(t p) w -> p t w", p=128))
            nc.sync.dma_start(out=sn4_t[:], in_=sn4_ext.ap().rearrange("(t p) w -> p t w", p=128))
            for c in range(CH):
                nc.sync.dma_start(out=bias_sb[c][:],
                                  in_=bias_ext[c].rearrange("t p k -> p t k"))

            # ---------------- phase 2+3: scale + inverse FFT ---------------
            for c in range(CH):
                # M = b_coef * global_amp_sum + bias   (in place in as_sb[c])
                nc.vector.scalar_tensor_tensor(
                    out=as_sb[c][:], in0=as_sb[c][:], scalar=float(b_coef),
                    in1=bias_sb[c][:], op0=AluOp.mult, op1=AluOp.add)

                for s in range(B_LOC):
                    img = c * B_LOC + s
                    yr = sb.tile([128, NT, KH], F32R, tag="yr", bufs=2)
                    yi = sb.tile([128, NT, KH], F32R, tag="yi", bufs=2)
                    nc.vector.tensor_mul(yr[:], e_r[img][:], as_sb[c][:])
                    nc.vector.tensor_mul(yi[:], e_i[img][:], as_sb[c][:])

                    p_r0 = sb.tile([128, N], F32R, tag="p_r0", bufs=2)
                    p_r1 = sb.tile([128, N], F32R, tag="p_r1", bufs=2)
                    p_i0 = sb.tile([128, N], F32R, tag="p_i0", bufs=2)
                    p_i1 = sb.tile([128, N], F32R, tag="p_i1", bufs=2)
                    for m3 in range(2):
                        mm = 128 if m3 == 0 else KT2
                        k2s = slice(m3 * 128, m3 * 128 + mm)
                        pr_ps = ps.tile([128, N], F32, tag="psA", name="pr_ps")
                        pi_ps = ps.tile([128, N], F32, tag="psB", name="pi_ps")
                        emit_group(
                            pr_ps[0:mm, :],
                            [(yr[:, k, k2s], c_t[:, k, :]) for k in range(NT)]
                            + [(yi[:, k, k2s], sn_t[:, k, :]) for k in range(NT)])
                        emit_group(
                            pi_ps[0:mm, :],
                            [(yi[:, k, k2s], c_t[:, k, :]) for k in range(NT)]
                            + [(yr[:, k, k2s], s_t[:, k, :]) for k in range(NT)])
                        pr_sb = p_r0 if m3 == 0 else p_r1
                        pi_sb = p_i0 if m3 == 0 else p_i1
                        nc.vector.tensor_copy(pr_sb[0:mm, :], pr_ps[0:mm, :])
                        nc.scalar.copy(pi_sb[0:mm, :], pi_ps[0:mm, :])

                    for mh in range(NT):
                        hs = slice(mh * 128, (mh + 1) * 128)
                        o_ps = ps.tile([128, N], F32, tag="psC", name="o_ps")
                        emit_group(
                            o_ps[:, :],
                            [(p_r0[:, hs], c4_t[:, 0, :]),
                             (p_r1[0:KT2, hs], c4_t[0:KT2, 1, :]),
                             (p_i0[:, hs], sn4_t[:, 0, :]),
                             (p_i1[0:KT2, hs], sn4_t[0:KT2, 1, :])])
                        out_sb = sb.tile([128, N], F32, tag="out_sb", bufs=3)
                        nc.scalar.copy(out_sb[:, :], o_ps[:, :])
                        nc.sync.dma_start(out=out_ext[s, c, hs, :], in_=out_sb[:, :])

    nc.compile()
    return nc


def _get_nc(b_coef: float):
    key = round(float(b_coef), 9)
    if key not in _COMPILED:
        _COMPILED[key] = _build(b_coef)
    return _COMPILED[key]


def kernel(x: np.ndarray, running_amp: np.ndarray) -> np.ndarray:
    x = np.ascontiguousarray(x, dtype=np.float32)
    running_amp = np.asarray(running_amp, dtype=np.float32)

    if float(np.sum(running_amp)) == 0.0:
        a_coef, b_coef = 0.0, 1.0
    else:
        a_coef, b_coef = 1.0 - MOMENTUM, MOMENTUM

    Cm, Sm, Snm, C4, Sn4 = _dft_constants()
    bias = (a_coef * running_amp[:, :, 0:KH]).reshape(CH, NT, 128, KH)
    bias = np.ascontiguousarray(bias, dtype=np.float32)

    nc = _get_nc(b_coef / B_FULL)

    in_maps = []
    for i in range(N_CORES):
        in_maps.append({
            "x": x[i * B_LOC:(i + 1) * B_LOC],
            "cm": Cm, "sm": Sm, "snm": Snm, "c4": C4, "sn4": Sn4,
            "bias": bias,
        })
    res = run_bass_kernel_spmd(nc, in_maps, list(range(N_CORES)))
    out = np.concatenate([res.results[i]["out"] for i in range(N_CORES)], axis=0)
    return out.astype(np.float32)
